# revision 34
# baseline (speedup 1.0000x reference)
"""KAN block (2x KAN layer, dense_mlp) TRN2 Bass kernel — data-parallel on 8 cores.

Full inputs in, full output out. Tokens (B*S = 4096) are sharded 8 ways
(512 per core); weights are replicated.

Device math per KAN layer (out = silu(x) @ wb.T + einsum('nig,oig->no', B(x), ws)):
each cubic B-spline on the uniform grid obeys the exact 2-term identity

    B_g(x) = [ (2 - |s_g|)+^3 - 4 (1 - |s_g|)+^3 ] / 6,   s_g = (x - c_g)/h

with c_g the center knot t_{g+2}. The 16 "tent-cube" features (p_g^3, q_g^3)
are bounded (<= 8), vanish outside the grid automatically (matching the
reference's zero extrapolation), and have near-zero cancellation in the
contraction — so both features and spline weights quantize to fp8e4 and the
spline matmuls run in DoubleRow mode (2 k-rows/cycle). The silu base path
stays fp32r. Weights carry a x256 scale so fp8 weights sit in the e4m3 sweet
range; the scale is undone for free in activation affine slots.

Layout: activations transposed (d on partitions, tokens on free dim).
Each tent-cube feature is ONE fused custom-DVE op (registered at import
time into concourse's custom-DVE table):

    TENT_CUBE_ANT: out = t^3,  t = min(|in0 - s0|*imm2 + s1, 0)   -> fp8e4

so a feature costs a single Vector pass (~0.6us/tile) instead of a 5-op
chain across ScalarE+VectorE. Matmul pairs (lhsT [128,2,128], rhs
[128,2,512]) accumulate with the fp32r base matmuls in the same PSUM group.
The only remaining ScalarE work is Silu and the final psum copies.
"""

import numpy as np
import ml_dtypes
from contextlib import ExitStack

import concourse.bass as bass
import concourse.bacc as bacc
import concourse.mybir as mybir
import concourse.tile as tile
import concourse.dve_ops as dve_ops
from concourse.bass_utils import run_bass_kernel_spmd
from concourse.dve_spec import (
    C0, C1, C2, AluOp as DveAlu, Bin, Spec, Src0, Zero, maxx, minn, sq,
)


def _register_tent_cube():
    """Custom fused DVE op: out = t^3, t = min(|in0 - s0|*imm2 + s1, 0).

    Computes a full tent-cube feature (-p^3 with p = (|s|+s1)_- clamp) in a
    single Vector pass, replacing a 5-op chain across two engines. The
    uops_sha is self-pinned at registration (lower() is deterministic within
    a process, which is all DveOp.compile's drift check needs)."""
    from concourse.dve_spec import lower
    from concourse.dve_uop import DveOpSpec

    name = "TENT_CUBE_ANT"
    if name in dve_ops._SUB_OPCODE_FOR_NAME:
        return next(op for op in dve_ops.OPS if op.name == name)
    d = Bin(DveAlu.SUBTRACT, Src0, C0)
    nd = Bin(DveAlu.SUBTRACT, C0, Src0)
    t = minn(Bin(DveAlu.ADD, Bin(DveAlu.MULTIPLY, maxx(d, nd), C2), C1), Zero)
    spec = Spec(
        body=Bin(DveAlu.MULTIPLY, sq(t), t),
        reference=lambda in0, in1, s0, s1, imm2: (
            np.minimum(np.abs(in0.astype(np.float32) - s0) * imm2 + s1,
                       0.0) ** 3
        ),
    )
    shas = {}
    for ver in ("v3", "v4"):
        try:
            shas[ver] = DveOpSpec(
                name=name, opcode=0, uops=lower(spec, ver=ver), rd1_en=False
            ).sha(ver)
        except Exception:
            pass
    op = dve_ops.DveOp(name, spec, subdim=False, uops_sha=shas)
    dve_ops.OPS.append(op)
    dve_ops._SUB_OPCODE_FOR_NAME[name] = (
        dve_ops._CUSTOM_DVE_ROW_BASE + len(dve_ops.OPS) - 1
    )
    assert dve_ops._SUB_OPCODE_FOR_NAME[name] < 0x20
    return op


TENT_CUBE = _register_tent_cube()

F32 = mybir.dt.float32
F32R = mybir.dt.float32r
BF16 = mybir.dt.bfloat16
FP8 = mybir.dt.float8e4
AF = mybir.ActivationFunctionType
ALU = mybir.AluOpType
DR = mybir.MatmulPerfMode.DoubleRow

# Problem constants (hardcoded per contract)
B, S, D, F = 2, 2048, 512, 2048
N_CORES = 8
T = (B * S) // N_CORES          # 512 tokens per core
G_INT = 5
H = 2.0 / G_INT                 # 0.4 knot spacing
NP = 8                          # 8 (p,q) tent pairs = 8 B-splines
NG1 = 4                         # layer-1 psum groups (4 m-tiles each)
D_T, F_T = D // 128, F // 128   # 4, 16
SC = 256.0                      # weight scale (psum carries SC*value)
CT = [float(g) - 3.5 for g in range(NP)]   # centers / h


def _e4(a):
    return np.clip(a, -240.0, 240.0).astype(ml_dtypes.float8_e4m3)


def pack_l1(w1b, w1s):
    """w1b (F,D), w1s (F,D,8) ->
       w1q (NG1, D_T, 128, NP, 2, 512) fp8, w1bt (NG1, D_T, 128, 512) f32."""
    A = np.asarray(w1s, np.float64).reshape(NG1, 4 * 128, D_T, 128, NP)
    A = A.transpose(0, 2, 3, 4, 1)                     # (gm, dt, i, g, o)
    w1q = _e4(np.stack([(-SC / 6.0) * A, (SC * 4.0 / 6.0) * A], axis=4))
    Wb = np.asarray(w1b, np.float64).reshape(NG1, 4 * 128, D_T, 128)
    Wb = (SC * Wb).transpose(0, 2, 3, 1)               # (gm, dt, i, o)
    return np.ascontiguousarray(w1q), np.ascontiguousarray(Wb.astype(np.float32))


def pack_l2(w2b, w2s):
    """w2b (D,F), w2s (D,F,8) ->
       w2q (F_T, 128, NP, 2, 512) fp8, w2bt (F_T, 128, 512) f32."""
    A = np.asarray(w2s, np.float64).reshape(D, F_T, 128, NP)
    A = A.transpose(1, 2, 3, 0)                        # (g2, i, g, o)
    w2q = _e4(np.stack([(-SC / 6.0) * A, (SC * 4.0 / 6.0) * A], axis=3))
    Wb = np.asarray(w2b, np.float64).reshape(D, F_T, 128)
    Wb = (SC * Wb).transpose(1, 2, 0)                  # (g2, i, o)
    return np.ascontiguousarray(w2q), np.ascontiguousarray(Wb.astype(np.float32))


def build_kernel():
    nc = bacc.Bacc()

    # const AP for the Silu activation's bias=0.0 operand
    ctens = nc.alloc_sbuf_tensor("const-zero", [128, 1], F32)
    nc.gpsimd.memset(ctens.ap(), 0.0)
    nc.const_aps.aps[(F32, 0.0)] = ctens.ap()
    nc.all_engine_barrier()
    # warmup ACT op: pulls the (one) activation table load to the very start
    # of the kernel instead of gating the first real Silu
    warm = nc.alloc_sbuf_tensor("act-warm", [128, 1], F32)
    nc.scalar.activation(warm.ap(), ctens.ap(), AF.Silu)

    xT = nc.declare_dram_parameter("xT", [D, T], F32, isOutput=False)
    w1q = nc.declare_dram_parameter("w1q", [NG1, D_T, 128, NP, 2, 512], FP8,
                                    isOutput=False)
    w1bt = nc.declare_dram_parameter("w1bt", [NG1, D_T, 128, 512], F32R,
                                     isOutput=False)
    w2q = nc.declare_dram_parameter("w2q", [F_T, 128, NP, 2, 512], FP8,
                                    isOutput=False)
    w2bt = nc.declare_dram_parameter("w2bt", [F_T, 128, 512], F32R,
                                     isOutput=False)
    outT = nc.declare_dram_parameter("outT", [D, T], F32, isOutput=True)

    with ExitStack() as ctx:
        tc = ctx.enter_context(tile.TileContext(nc))
        xp = ctx.enter_context(tc.tile_pool(name="xp", bufs=1))
        f1p = ctx.enter_context(tc.tile_pool(name="f1p", bufs=1))
        f2p = ctx.enter_context(tc.tile_pool(name="f2p", bufs=1))
        scr = ctx.enter_context(tc.tile_pool(name="scr", bufs=7))
        w1pool = ctx.enter_context(tc.tile_pool(name="w1p", bufs=3))
        w2pool = ctx.enter_context(tc.tile_pool(name="w2p", bufs=3))
        opool = ctx.enter_context(tc.tile_pool(name="op", bufs=4))
        pp = ctx.enter_context(tc.tile_pool(name="pp", bufs=1, space="PSUM"))

        def gen_sil_xb(src, fpool, blk, siltag, l2, silbufs=1):
            """Phase 1: the only two reads of src (PSUM for l2) — frees the
            psum bank as early as possible."""
            sil = fpool.tile([128, T], F32R, name=f"sil{blk}", tag=siltag,
                             bufs=silbufs)
            nc.scalar.activation(sil, src, AF.Silu,
                                 scale=(1.0 / SC) if l2 else 1.0)
            if l2:
                xb = scr.tile([128, T], BF16, name=f"xb{blk}", tag="xb", bufs=5)
                nc.vector.tensor_scalar(out=xb, in0=src,
                                        scalar1=1.0 / (SC * H), scalar2=None,
                                        op0=ALU.mult)
                return sil, xb
            return sil, src

        def gen_tents(tsrc, fpool, blk, ftagpfx, l2, fbufs=1):
            """Phase 2: 16 fused tent-cube ops -> 8 fp8 DoubleRow pair tiles."""
            s0s, inv = (CT, 1.0) if l2 else ([c * H for c in CT], 1.0 / H)
            fpairs = []
            for g in range(NP):
                fp = fpool.tile([128, 2, T], FP8, name=f"f{blk}_{g}",
                                tag=f"{ftagpfx}_{g}", bufs=fbufs)
                nc.vector._custom_dve(TENT_CUBE, out=fp[:, 0, :], in0=tsrc,
                                      s0=float(s0s[g]), s1=-2.0, imm2=inv)
                nc.vector._custom_dve(TENT_CUBE, out=fp[:, 1, :], in0=tsrc,
                                      s0=float(s0s[g]), s1=-1.0, imm2=inv)
                fpairs.append(fp)
            return fpairs

        def gen_features(src, fpool, blk, siltag, ftagpfx, l2, silbufs=1,
                         fbufs=1):
            sil, tsrc = gen_sil_xb(src, fpool, blk, siltag, l2, silbufs)
            return sil, gen_tents(tsrc, fpool, blk, ftagpfx, l2, fbufs)

        # ---- load x, generate layer-1 features (once) ----
        # DMA issue order matters at the head: x[0] (gates sil1[0]) and the
        # small gm0 base weights (gate the first matmul) go before the rest.
        xtiles = [xp.tile([128, T], F32, name=f"x{dt}", tag=f"x{dt}")
                  for dt in range(D_T)]
        wbs0 = [w1pool.tile([128, 512], F32R, name=f"w1b_0_{dt}",
                            tag="w1b", bufs=8) for dt in range(D_T)]
        nc.sync.dma_start(out=xtiles[0], in_=xT[0:128, :])
        nc.sync.dma_start(out=wbs0[0], in_=w1bt[0, 0])
        for dt in range(1, D_T):
            nc.sync.dma_start(out=xtiles[dt],
                              in_=xT[dt * 128:(dt + 1) * 128, :])
        for dt in range(1, D_T):
            nc.sync.dma_start(out=wbs0[dt], in_=w1bt[0, dt])

        sil1, f1 = [], []
        for dt in range(D_T):
            s, fp = gen_features(xtiles[dt], f1p, blk=f"a{dt}",
                                 siltag=f"sil1_{dt}", ftagpfx=f"f1_{dt}",
                                 l2=False)
            sil1.append(s)
            f1.append(fp)

        psum2 = [pp.tile([128, T], F32, name=f"ps2_{m2}", tag=f"ps2_{m2}")
                 for m2 in range(D_T)]

        def emit_l1(gm, wbs=None):
            ps = [pp.tile([128, T], F32, name=f"ps1_{gm}_{mi}", tag=f"ps1_{mi}")
                  for mi in range(4)]
            if wbs is None:
                wbs = []
                for dt in range(D_T):
                    wb = w1pool.tile([128, 512], F32R, name=f"w1b_{gm}_{dt}",
                                     tag="w1b", bufs=8)
                    nc.sync.dma_start(out=wb, in_=w1bt[gm, dt])
                    wbs.append(wb)
            for dt in range(D_T):
                for mi in range(4):
                    nc.tensor.matmul(ps[mi],
                                     lhsT=wbs[dt][:, mi * 128:(mi + 1) * 128],
                                     rhs=sil1[dt], start=(dt == 0), stop=False)
                wq = w1pool.tile([128, NP, 2, 512], FP8, name=f"w1q_{gm}_{dt}",
                                 tag="w1q")
                nc.sync.dma_start(out=wq, in_=w1q[gm, dt])
                for g in range(NP):
                    for mi in range(4):
                        nc.tensor.matmul(
                            ps[mi],
                            lhsT=wq[:, g, :, mi * 128:(mi + 1) * 128],
                            rhs=f1[dt][g], perf_mode=DR,
                            start=False,
                            stop=(dt == D_T - 1 and g == NP - 1))
            return ps

        def emit_l2(gm, sil2, f2):
            for mi in range(4):
                g2 = gm * 4 + mi
                wb = w2pool.tile([128, 512], F32R, name=f"w2b_{g2}", tag="w2b")
                nc.sync.dma_start(out=wb, in_=w2bt[g2])
                for m2 in range(D_T):
                    nc.tensor.matmul(psum2[m2],
                                     lhsT=wb[:, m2 * 128:(m2 + 1) * 128],
                                     rhs=sil2[mi], start=(g2 == 0), stop=False)
                wq = w2pool.tile([128, NP, 2, 512], FP8, name=f"w2q_{g2}",
                                 tag="w2q")
                nc.sync.dma_start(out=wq, in_=w2q[g2])
                for g in range(NP):
                    for m2 in range(D_T):
                        nc.tensor.matmul(
                            psum2[m2],
                            lhsT=wq[:, g, :, m2 * 128:(m2 + 1) * 128],
                            rhs=f2[mi][g], perf_mode=DR,
                            start=False,
                            stop=(g2 == F_T - 1 and g == NP - 1))

        # ---- main pipeline: L1(gm) matmuls || L2 feature-gen || L2 matmuls ----
        psum1 = emit_l1(0, wbs=wbs0)
        for gm in range(NG1):
            sil2, xb2, f2 = [], [], []
            for mi in range(4):
                g2 = gm * 4 + mi
                s, xb = gen_sil_xb(psum1[mi], f2p, blk=f"b{g2}",
                                   siltag=f"sil2_{mi}", l2=True, silbufs=2)
                sil2.append(s)
                xb2.append(xb)
            for mi in range(4):
                g2 = gm * 4 + mi
                f2.append(gen_tents(xb2[mi], f2p, blk=f"b{g2}",
                                    ftagpfx=f"f2_{mi}", l2=True))
            if gm < NG1 - 1:
                psum1 = emit_l1(gm + 1)
            emit_l2(gm, sil2, f2)

        for m2 in range(D_T):
            ot = opool.tile([128, T], F32, name=f"o{m2}", tag="out")
            nc.scalar.activation(ot, psum2[m2], AF.Copy, scale=1.0 / SC)
            nc.sync.dma_start(out=outT[m2 * 128:(m2 + 1) * 128, :], in_=ot)

    nc.finalize()
    return nc


_NC_CACHE = None


def _get_nc():
    global _NC_CACHE
    if _NC_CACHE is None:
        _NC_CACHE = build_kernel()
    return _NC_CACHE


def run(x, w1_base, w1_spline, w2_base, w2_spline, trace=False, **spmd_kwargs):
    x = np.asarray(x, dtype=np.float32)
    xf = np.ascontiguousarray(x.reshape(B * S, D))
    w1qa, w1ba = pack_l1(np.asarray(w1_base), np.asarray(w1_spline))
    w2qa, w2ba = pack_l2(np.asarray(w2_base), np.asarray(w2_spline))
    in_maps = []
    for c in range(N_CORES):
        shard = xf[c * T:(c + 1) * T]
        in_maps.append({
            "xT": np.ascontiguousarray(shard.T),
            "w1q": w1qa,
            "w1bt": w1ba,
            "w2q": w2qa,
            "w2bt": w2ba,
        })
    nc = _get_nc()
    res = run_bass_kernel_spmd(nc, in_maps, list(range(N_CORES)),
                               trace=trace, **spmd_kwargs)
    outs = [np.asarray(r["outT"]).T for r in res.results]   # each (T, D)
    out = np.concatenate(outs, axis=0).reshape(B, S, D).astype(np.float32)
    return out, res


def kernel(x, grid, w1_base, w1_spline, w2_base, w2_spline):
    out, _ = run(x, w1_base, w1_spline, w2_base, w2_spline)
    return out


# revision 37
# speedup vs baseline: 1.0057x; 1.0057x over previous
"""KAN block (2x KAN layer, dense_mlp) TRN2 Bass kernel — data-parallel on 8 cores.

Full inputs in, full output out. Tokens (B*S = 4096) are sharded 8 ways
(512 per core); weights are replicated.

Device math per KAN layer (out = silu(x) @ wb.T + einsum('nig,oig->no', B(x), ws)):
each cubic B-spline on the uniform grid obeys the exact 2-term identity

    B_g(x) = [ (2 - |s_g|)+^3 - 4 (1 - |s_g|)+^3 ] / 6,   s_g = (x - c_g)/h

with c_g the center knot t_{g+2}. The 16 "tent-cube" features (p_g^3, q_g^3)
are bounded (<= 8), vanish outside the grid automatically (matching the
reference's zero extrapolation), and have near-zero cancellation in the
contraction — so both features and spline weights quantize to fp8e4 and the
spline matmuls run in DoubleRow mode (2 k-rows/cycle). The silu base path
stays fp32r. Weights carry a x256 scale so fp8 weights sit in the e4m3 sweet
range; the scale is undone for free in activation affine slots.

Layout: activations transposed (d on partitions, tokens on free dim).
Each tent-cube feature is ONE fused custom-DVE op (registered at import
time into concourse's custom-DVE table):

    TENT_CUBE_ANT: out = t^3,  t = min(|in0 - s0|*imm2 + s1, 0)   -> fp8e4

so a feature costs a single Vector pass (~0.6us/tile) instead of a 5-op
chain across ScalarE+VectorE. Matmul pairs (lhsT [128,2,128], rhs
[128,2,512]) accumulate with the fp32r base matmuls in the same PSUM group.
The only remaining ScalarE work is Silu and the final psum copies.
"""

import numpy as np
import ml_dtypes
from contextlib import ExitStack

import concourse.bass as bass
import concourse.bacc as bacc
import concourse.mybir as mybir
import concourse.tile as tile
import concourse.dve_ops as dve_ops
from concourse.bass_utils import run_bass_kernel_spmd
from concourse.dve_spec import (
    C0, C1, C2, AluOp as DveAlu, Bin, Spec, Src0, Zero, maxx, minn, sq,
)


def _register_tent_cube():
    """Custom fused DVE op: out = t^3, t = min(|in0 - s0|*imm2 + s1, 0).

    Computes a full tent-cube feature (-p^3 with p = (|s|+s1)_- clamp) in a
    single Vector pass, replacing a 5-op chain across two engines. The
    uops_sha is self-pinned at registration (lower() is deterministic within
    a process, which is all DveOp.compile's drift check needs)."""
    from concourse.dve_spec import lower
    from concourse.dve_uop import DveOpSpec

    name = "TENT_CUBE_ANT"
    if name in dve_ops._SUB_OPCODE_FOR_NAME:
        return next(op for op in dve_ops.OPS if op.name == name)
    d = Bin(DveAlu.SUBTRACT, Src0, C0)
    nd = Bin(DveAlu.SUBTRACT, C0, Src0)
    t = minn(Bin(DveAlu.ADD, Bin(DveAlu.MULTIPLY, maxx(d, nd), C2), C1), Zero)
    spec = Spec(
        body=Bin(DveAlu.MULTIPLY, sq(t), t),
        reference=lambda in0, in1, s0, s1, imm2: (
            np.minimum(np.abs(in0.astype(np.float32) - s0) * imm2 + s1,
                       0.0) ** 3
        ),
    )
    shas = {}
    for ver in ("v3", "v4"):
        try:
            shas[ver] = DveOpSpec(
                name=name, opcode=0, uops=lower(spec, ver=ver), rd1_en=False
            ).sha(ver)
        except Exception:
            pass
    op = dve_ops.DveOp(name, spec, subdim=False, uops_sha=shas)
    dve_ops.OPS.append(op)
    dve_ops._SUB_OPCODE_FOR_NAME[name] = (
        dve_ops._CUSTOM_DVE_ROW_BASE + len(dve_ops.OPS) - 1
    )
    assert dve_ops._SUB_OPCODE_FOR_NAME[name] < 0x20
    return op


TENT_CUBE = _register_tent_cube()

F32 = mybir.dt.float32
F32R = mybir.dt.float32r
BF16 = mybir.dt.bfloat16
FP8 = mybir.dt.float8e4
AF = mybir.ActivationFunctionType
ALU = mybir.AluOpType
DR = mybir.MatmulPerfMode.DoubleRow

# Problem constants (hardcoded per contract)
B, S, D, F = 2, 2048, 512, 2048
N_CORES = 8
T = (B * S) // N_CORES          # 512 tokens per core
G_INT = 5
H = 2.0 / G_INT                 # 0.4 knot spacing
NP = 8                          # 8 (p,q) tent pairs = 8 B-splines
NG1 = 4                         # layer-1 psum groups (4 m-tiles each)
D_T, F_T = D // 128, F // 128   # 4, 16
SC = 256.0                      # weight scale (psum carries SC*value)
CT = [float(g) - 3.5 for g in range(NP)]   # centers / h


def _e4(a):
    return np.clip(a, -240.0, 240.0).astype(ml_dtypes.float8_e4m3)


def pack_l1(w1b, w1s):
    """w1b (F,D), w1s (F,D,8) ->
       w1q (NG1, D_T, 128, NP, 2, 512) fp8, w1bt (NG1, D_T, 128, 512) f32."""
    A = np.asarray(w1s, np.float64).reshape(NG1, 4 * 128, D_T, 128, NP)
    A = A.transpose(0, 2, 3, 4, 1)                     # (gm, dt, i, g, o)
    w1q = _e4(np.stack([(-SC / 6.0) * A, (SC * 4.0 / 6.0) * A], axis=4))
    Wb = np.asarray(w1b, np.float64).reshape(NG1, 4 * 128, D_T, 128)
    Wb = (SC * Wb).transpose(0, 2, 3, 1)               # (gm, dt, i, o)
    return np.ascontiguousarray(w1q), np.ascontiguousarray(Wb.astype(np.float32))


def pack_l2(w2b, w2s):
    """w2b (D,F), w2s (D,F,8) ->
       w2q (F_T, 128, NP, 2, 512) fp8, w2bt (F_T, 128, 512) f32."""
    A = np.asarray(w2s, np.float64).reshape(D, F_T, 128, NP)
    A = A.transpose(1, 2, 3, 0)                        # (g2, i, g, o)
    w2q = _e4(np.stack([(-SC / 6.0) * A, (SC * 4.0 / 6.0) * A], axis=3))
    Wb = np.asarray(w2b, np.float64).reshape(D, F_T, 128)
    Wb = (SC * Wb).transpose(1, 2, 0)                  # (g2, i, o)
    return np.ascontiguousarray(w2q), np.ascontiguousarray(Wb.astype(np.float32))


def build_kernel():
    nc = bacc.Bacc()

    # const AP for the Silu activation's bias=0.0 operand
    ctens = nc.alloc_sbuf_tensor("const-zero", [128, 1], F32)
    nc.gpsimd.memset(ctens.ap(), 0.0)
    nc.const_aps.aps[(F32, 0.0)] = ctens.ap()
    nc.all_engine_barrier()
    # warmup ACT op: pulls the (one) activation table load to the very start
    # of the kernel instead of gating the first real Silu
    warm = nc.alloc_sbuf_tensor("act-warm", [128, 1], F32)
    nc.scalar.activation(warm.ap(), ctens.ap(), AF.Silu)

    xT = nc.declare_dram_parameter("xT", [D, T], F32, isOutput=False)
    w1q = nc.declare_dram_parameter("w1q", [NG1, D_T, 128, NP, 2, 512], FP8,
                                    isOutput=False)
    w1bt = nc.declare_dram_parameter("w1bt", [NG1, D_T, 128, 512], F32R,
                                     isOutput=False)
    w2q = nc.declare_dram_parameter("w2q", [F_T, 128, NP, 2, 512], FP8,
                                    isOutput=False)
    w2bt = nc.declare_dram_parameter("w2bt", [F_T, 128, 512], F32R,
                                     isOutput=False)
    outT = nc.declare_dram_parameter("outT", [D, T], F32, isOutput=True)

    with ExitStack() as ctx:
        tc = ctx.enter_context(tile.TileContext(nc))
        xp = ctx.enter_context(tc.tile_pool(name="xp", bufs=1))
        f1p = ctx.enter_context(tc.tile_pool(name="f1p", bufs=1))
        f2p = ctx.enter_context(tc.tile_pool(name="f2p", bufs=1))
        scr = ctx.enter_context(tc.tile_pool(name="scr", bufs=7))
        w1pool = ctx.enter_context(tc.tile_pool(name="w1p", bufs=3))
        w2pool = ctx.enter_context(tc.tile_pool(name="w2p", bufs=3))
        opool = ctx.enter_context(tc.tile_pool(name="op", bufs=4))
        pp = ctx.enter_context(tc.tile_pool(name="pp", bufs=1, space="PSUM"))

        def gen_sil_xb(src, fpool, blk, siltag, l2, silbufs=1):
            """Phase 1: the only two reads of src (PSUM for l2) — frees the
            psum bank as early as possible."""
            sil = fpool.tile([128, T], F32R, name=f"sil{blk}", tag=siltag,
                             bufs=silbufs)
            nc.scalar.activation(sil, src, AF.Silu,
                                 scale=(1.0 / SC) if l2 else 1.0)
            if l2:
                xb = scr.tile([128, T], BF16, name=f"xb{blk}", tag="xb", bufs=5)
                nc.vector.tensor_scalar(out=xb, in0=src,
                                        scalar1=1.0 / (SC * H), scalar2=None,
                                        op0=ALU.mult)
                return sil, xb
            return sil, src

        def gen_tents(tsrc, fpool, blk, ftagpfx, l2, fbufs=1):
            """Phase 2: 16 fused tent-cube ops -> 8 fp8 DoubleRow pair tiles."""
            s0s, inv = (CT, 1.0) if l2 else ([c * H for c in CT], 1.0 / H)
            fpairs = []
            for g in range(NP):
                fp = fpool.tile([128, 2, T], FP8, name=f"f{blk}_{g}",
                                tag=f"{ftagpfx}_{g}", bufs=fbufs)
                nc.vector._custom_dve(TENT_CUBE, out=fp[:, 0, :], in0=tsrc,
                                      s0=float(s0s[g]), s1=-2.0, imm2=inv)
                nc.vector._custom_dve(TENT_CUBE, out=fp[:, 1, :], in0=tsrc,
                                      s0=float(s0s[g]), s1=-1.0, imm2=inv)
                fpairs.append(fp)
            return fpairs

        def gen_features(src, fpool, blk, siltag, ftagpfx, l2, silbufs=1,
                         fbufs=1):
            sil, tsrc = gen_sil_xb(src, fpool, blk, siltag, l2, silbufs)
            return sil, gen_tents(tsrc, fpool, blk, ftagpfx, l2, fbufs)

        # ---- load x, generate layer-1 features (once) ----
        # DMA issue order matters at the head: x[0] (gates sil1[0]) and the
        # small gm0 base weights (gate the first matmul) go before the rest.
        xtiles = [xp.tile([128, T], F32, name=f"x{dt}", tag=f"x{dt}")
                  for dt in range(D_T)]
        wbs0 = [w1pool.tile([128, 512], F32R, name=f"w1b_0_{dt}",
                            tag="w1b", bufs=8) for dt in range(D_T)]
        nc.sync.dma_start(out=xtiles[0], in_=xT[0:128, :])
        nc.sync.dma_start(out=wbs0[0], in_=w1bt[0, 0])
        for dt in range(1, D_T):
            nc.sync.dma_start(out=xtiles[dt],
                              in_=xT[dt * 128:(dt + 1) * 128, :])
            nc.sync.dma_start(out=wbs0[dt], in_=w1bt[0, dt])

        sil1, f1 = [], []
        for dt in range(D_T):
            s, fp = gen_features(xtiles[dt], f1p, blk=f"a{dt}",
                                 siltag=f"sil1_{dt}", ftagpfx=f"f1_{dt}",
                                 l2=False)
            sil1.append(s)
            f1.append(fp)

        psum2 = [pp.tile([128, T], F32, name=f"ps2_{m2}", tag=f"ps2_{m2}")
                 for m2 in range(D_T)]

        def emit_l1(gm, wbs=None):
            ps = [pp.tile([128, T], F32, name=f"ps1_{gm}_{mi}", tag=f"ps1_{mi}")
                  for mi in range(4)]
            if wbs is None:
                wbs = []
                for dt in range(D_T):
                    wb = w1pool.tile([128, 512], F32R, name=f"w1b_{gm}_{dt}",
                                     tag="w1b", bufs=8)
                    nc.sync.dma_start(out=wb, in_=w1bt[gm, dt])
                    wbs.append(wb)
            for dt in range(D_T):
                for mi in range(4):
                    nc.tensor.matmul(ps[mi],
                                     lhsT=wbs[dt][:, mi * 128:(mi + 1) * 128],
                                     rhs=sil1[dt], start=(dt == 0), stop=False)
            for dt in range(D_T):
                wq = w1pool.tile([128, NP, 2, 512], FP8, name=f"w1q_{gm}_{dt}",
                                 tag="w1q")
                nc.sync.dma_start(out=wq, in_=w1q[gm, dt])
                for g in range(NP):
                    for mi in range(4):
                        nc.tensor.matmul(
                            ps[mi],
                            lhsT=wq[:, g, :, mi * 128:(mi + 1) * 128],
                            rhs=f1[dt][g], perf_mode=DR,
                            start=False,
                            stop=(dt == D_T - 1 and g == NP - 1))
            return ps

        def emit_l2(gm, sil2, f2):
            for mi in range(4):
                g2 = gm * 4 + mi
                wb = w2pool.tile([128, 512], F32R, name=f"w2b_{g2}", tag="w2b")
                nc.sync.dma_start(out=wb, in_=w2bt[g2])
                for m2 in range(D_T):
                    nc.tensor.matmul(psum2[m2],
                                     lhsT=wb[:, m2 * 128:(m2 + 1) * 128],
                                     rhs=sil2[mi], start=(g2 == 0), stop=False)
                wq = w2pool.tile([128, NP, 2, 512], FP8, name=f"w2q_{g2}",
                                 tag="w2q")
                nc.sync.dma_start(out=wq, in_=w2q[g2])
                for g in range(NP):
                    for m2 in range(D_T):
                        nc.tensor.matmul(
                            psum2[m2],
                            lhsT=wq[:, g, :, m2 * 128:(m2 + 1) * 128],
                            rhs=f2[mi][g], perf_mode=DR,
                            start=False,
                            stop=(g2 == F_T - 1 and g == NP - 1))

        # ---- main pipeline: L1(gm) matmuls || L2 feature-gen || L2 matmuls ----
        psum1 = emit_l1(0, wbs=wbs0)
        for gm in range(NG1):
            sil2, xb2, f2 = [], [], []
            for mi in range(4):
                g2 = gm * 4 + mi
                s, xb = gen_sil_xb(psum1[mi], f2p, blk=f"b{g2}",
                                   siltag=f"sil2_{mi}", l2=True, silbufs=2)
                sil2.append(s)
                xb2.append(xb)
            for mi in range(4):
                g2 = gm * 4 + mi
                f2.append(gen_tents(xb2[mi], f2p, blk=f"b{g2}",
                                    ftagpfx=f"f2_{mi}", l2=True))
            if gm < NG1 - 1:
                psum1 = emit_l1(gm + 1)
            emit_l2(gm, sil2, f2)

        for m2 in range(D_T):
            ot = opool.tile([128, T], F32, name=f"o{m2}", tag="out")
            nc.scalar.activation(ot, psum2[m2], AF.Copy, scale=1.0 / SC)
            nc.sync.dma_start(out=outT[m2 * 128:(m2 + 1) * 128, :], in_=ot)

    nc.finalize()
    return nc


_NC_CACHE = None


def _get_nc():
    global _NC_CACHE
    if _NC_CACHE is None:
        _NC_CACHE = build_kernel()
    return _NC_CACHE


def run(x, w1_base, w1_spline, w2_base, w2_spline, trace=False, **spmd_kwargs):
    x = np.asarray(x, dtype=np.float32)
    xf = np.ascontiguousarray(x.reshape(B * S, D))
    w1qa, w1ba = pack_l1(np.asarray(w1_base), np.asarray(w1_spline))
    w2qa, w2ba = pack_l2(np.asarray(w2_base), np.asarray(w2_spline))
    in_maps = []
    for c in range(N_CORES):
        shard = xf[c * T:(c + 1) * T]
        in_maps.append({
            "xT": np.ascontiguousarray(shard.T),
            "w1q": w1qa,
            "w1bt": w1ba,
            "w2q": w2qa,
            "w2bt": w2ba,
        })
    nc = _get_nc()
    res = run_bass_kernel_spmd(nc, in_maps, list(range(N_CORES)),
                               trace=trace, **spmd_kwargs)
    outs = [np.asarray(r["outT"]).T for r in res.results]   # each (T, D)
    out = np.concatenate(outs, axis=0).reshape(B, S, D).astype(np.float32)
    return out, res


def kernel(x, grid, w1_base, w1_spline, w2_base, w2_spline):
    out, _ = run(x, w1_base, w1_spline, w2_base, w2_spline)
    return out
